# revision 30
# baseline (speedup 1.0000x reference)
"""Trainium2 Bass kernel for nn_Attn_14078902796904.

Computes attn = softmax(encoder_outputs @ hidden) for
encoder_outputs [65536, 1024] f32, hidden [1024] f32 -> [1, 1, 65536] f32.

Strategy (sequence-parallel across 8 NeuronCores):
  - Core c gets rows [c*8192, (c+1)*8192) of encoder_outputs; hidden is
    replicated (host pre-broadcasts it to [128, 1024]).
  - On-core: stream the 32 MB shard through SBUF in [128, nb*1024]
    chunks with a CONTIGUOUS per-partition layout (partition p of a
    chunk holds nb consecutive rows -> 16 KB contiguous HBM reads per
    partition, 16 KB DMA descriptors, ~405 GB/s observed).
    All chunks keep the full 128-partition shape: partition-sliced DMAs
    collapse onto 4 of the 16 SDMA rings (~1/4 bandwidth -- measured).
  - Compute: the DVE multiplies each chunk by hid IN PLACE (f32
    tensor_tensor is 1 elem/lane/cycle -> 4.42 us per 4-block chunk vs
    the 5.2-5.9 us DMA pace).  Per-block row-sums are split between the
    Scalar engine (activation Identity + accum_out, 1.22 us/block
    sustained) and the DVE (reduce_sum, 1.22 us/block): scalar takes
    most blocks, the DVE takes a few mid-stream ones plus the tapered
    tail blocks interleaved with its final muls, so neither engine
    holds a backlog when the stream ends.
  - Device returns raw energies; softmax runs on the host in float64
    over all 65536 gathered energies.  No on-device softmax -> no
    ACT_TABLE_LOAD, minimal post-stream tail.
  (tensor_tensor_reduce would fuse mul+reduce in one DVE op but crashes
  the execution unit on this runtime path -- probed 2026-08-08.)
"""

import os
import sys
import time

for _p in ("/opt/trn_rl_repo", "/root/.axon_site/_ro/trn_rl_repo"):
    if os.path.isdir(_p) and _p not in sys.path:
        sys.path.append(_p)

import numpy as np

import concourse.tile as tile
from concourse import bacc, mybir
from concourse.bass_utils import run_bass_kernel_spmd

S = 65536
H = 1024
N_CORES = 8
SC = S // N_CORES          # 8192 rows per core
P = 128                    # partitions
NT = SC // P               # 64 blocks of 128 rows per core
GMAX = 4                   # max blocks per DMA chunk (2 MB; 8-block
                           # chunks hit a DVE efficiency cliff)

# chunk sizes in blocks; tapered at the end (shorter post-DMA tail)
CHUNKS = [4] * 14 + [2, 2, 2, 1, 1]
assert sum(CHUNKS) == NT

INP_BUFS = 7

_DT = mybir.dt.float32


def _vector_blocks(g, nb):
    """Block offsets (within chunk g) computed entirely on the DVE by a
    fused scalar_tensor_tensor (mul+accum in ONE 1.25us pass -- the
    only fused mul+reduce that works on this runtime; the TTR op
    crashes).  All other blocks: DVE multiply + Scalar-engine row-sum.
    STT owns the taper so the endgame has no cross-engine handoff."""
    if nb == 4:
        return [3] if g in (3, 9) else []
    if nb == 2:
        return [1]            # taper 2-block chunks: [scalar, STT]
    return [0]                # final 1-block chunks: STT


def _build_nc():
    nc = bacc.Bacc("TRN2", target_bir_lowering=False, debug=False,
                   enable_asserts=False, num_devices=N_CORES)
    enc = nc.dram_tensor("enc", [SC, H], _DT, kind="ExternalInput")
    hid = nc.dram_tensor("hid", [P, H], _DT, kind="ExternalInput")
    n_v = sum(len(_vector_blocks(g, nb)) for g, nb in enumerate(CHUNKS))
    out_s = nc.dram_tensor("out_s", [P, NT], _DT, kind="ExternalOutput")
    out_v = nc.dram_tensor("out_v", [P, n_v], _DT, kind="ExternalOutput")

    with tile.TileContext(nc) as tc:
        with (
            tc.tile_pool(name="inp", bufs=INP_BUFS) as inp_pool,
            tc.tile_pool(name="small", bufs=1) as small,
        ):
            hidrep = small.tile([P, H], _DT)

            # separate tiles per engine so the Tile tracker never makes
            # one engine's energy writes wait on the other's
            energies_s = small.tile([P, NT], _DT)
            energies_v = small.tile([P, max(n_v, 1)], _DT)
            # the DVE-reduced columns of energies_s are never written on
            # device (host overwrites them from out_v); zero once so the
            # final out_s DMA reads initialized memory
            nc.gpsimd.memset(energies_s[:], 0.0)

            vcol = 0
            blk = 0
            for g, nb in enumerate(CHUNKS):
                r0 = blk * P
                t_in = inp_pool.tile([P, GMAX * H], _DT, tag="t_in")
                # partition p <- rows [r0 + p*nb, r0 + (p+1)*nb):
                # 4*nb KB contiguous per partition.  All chunk DMAs
                # issue from the Sync sequencer: scalar-issued DMAs
                # serialize behind the Scalar engine's ACTIVATE queue
                # and starve the stream (measured +15 us).
                nc.sync.dma_start(
                    t_in[:, :nb * H].rearrange("p (b h) -> p b h", h=H),
                    enc.ap()[r0:r0 + nb * P, :].rearrange(
                        "(p b) h -> p b h", p=P),
                )
                if g == 0:
                    # issued after chunk 0 so the big stream's first
                    # byte isn't delayed behind the hid replica
                    nc.scalar.dma_start(hidrep[:], hid.ap())  # scalar is idle during ramp
                vblocks = _vector_blocks(g, nb)
                n_mul = nb - len(vblocks)   # STT blocks are the last ones
                # multiply scalar-owned blocks in <=2-block pieces: the
                # Scalar engine's row-sums for the early blocks start
                # ~2 us sooner than with one whole-chunk mul
                for m0 in range(0, n_mul, 2):
                    mb = min(2, n_mul - m0)
                    hid_bc = hidrep[:].rearrange(
                        "p (o h) -> p o h", o=1).broadcast_to((P, mb, H))
                    nc.vector.tensor_mul(
                        t_in[:, m0 * H:(m0 + mb) * H].rearrange(
                            "p (b h) -> p b h", h=H),
                        t_in[:, m0 * H:(m0 + mb) * H].rearrange(
                            "p (b h) -> p b h", h=H),
                        hid_bc,
                    )
                for j in range(nb):
                    seg = t_in[:, j * H:(j + 1) * H]
                    if j in vblocks:
                        # fused mul+accum on the DVE, one pass
                        nc.vector.scalar_tensor_tensor(
                            seg, seg, 1.0, hidrep[:],
                            op0=mybir.AluOpType.mult,
                            op1=mybir.AluOpType.mult,
                            accum_out=energies_v[:, vcol:vcol + 1],
                        )
                        vcol += 1
                    else:
                        nc.scalar.activation(
                            seg, seg,
                            mybir.ActivationFunctionType.Identity,
                            accum_out=energies_s[:, blk + j:blk + j + 1],
                        )
                blk += nb

            nc.sync.dma_start(out_s.ap(), energies_s[:])
            nc.sync.dma_start(out_v.ap(), energies_v[:, :n_v])
    nc.compile()
    return nc


_NC_CACHE = None


def _get_nc():
    global _NC_CACHE
    if _NC_CACHE is None:
        _NC_CACHE = _build_nc()
    return _NC_CACHE


def run_device(hidden, encoder_outputs, **spmd_kwargs):
    """Run the per-core kernels; returns (list of per-core result dicts,
    BassKernelResults)."""
    hidden = np.asarray(hidden, dtype=np.float32)
    encoder_outputs = np.asarray(encoder_outputs, dtype=np.float32)
    hidrep = np.ascontiguousarray(np.broadcast_to(hidden, (P, H)))
    in_maps = [
        {
            "enc": np.ascontiguousarray(encoder_outputs[c * SC:(c + 1) * SC]),
            "hid": hidrep,
        }
        for c in range(N_CORES)
    ]
    # The axon-proxied runtime occasionally reports the accelerator as
    # unrecoverable and then recovers on the next attempt; retry.
    last_err = None
    for attempt in range(3):
        try:
            res = run_bass_kernel_spmd(
                _get_nc(), in_maps, list(range(N_CORES)), **spmd_kwargs
            )
            return res.results, res
        except Exception as e:  # noqa: BLE001
            last_err = e
            time.sleep(2.0)
    raise last_err


def _maps():
    """(vcols, perm): vcols[i] = global block column of the i-th DVE
    reduce; perm[s_local] = flat index into the merged [P, NT] energies."""
    vcols = []
    blk = 0
    for g, nb in enumerate(CHUNKS):
        for j in _vector_blocks(g, nb):
            vcols.append(blk + j)
        blk += nb
    perm = np.empty(SC, dtype=np.int64)
    blk = 0
    for nb in CHUNKS:
        r0 = blk * P
        for p in range(P):
            base = r0 + p * nb
            for j in range(nb):
                perm[base + j] = p * NT + (blk + j)
        blk += nb
    return np.array(vcols), perm


_VCOLS, _PERM = _maps()


def combine(results):
    """Host-side softmax over the gathered energies -> [1, 1, S] f32."""
    es = []
    for r in results:
        E = r["out_s"].copy()
        E[:, _VCOLS] = r["out_v"]
        es.append(E.reshape(P * NT)[_PERM])
    e = np.concatenate(es).astype(np.float64)
    e -= e.max()
    x = np.exp(e)
    attn = x / x.sum()
    return attn.astype(np.float32)[None, None, :]


def kernel(hidden, encoder_outputs):
    results, _ = run_device(hidden, encoder_outputs)
    return combine(results)


# revision 34
# speedup vs baseline: 1.1577x; 1.1577x over previous
"""Trainium2 Bass kernel for nn_Attn_14078902796904.

Computes attn = softmax(encoder_outputs @ hidden) for
encoder_outputs [65536, 1024] f32, hidden [1024] f32 -> [1, 1, 65536] f32.

Strategy (sequence-parallel across 8 NeuronCores):
  - Core c gets rows [c*8192, (c+1)*8192) of encoder_outputs; hidden
    arrives as one 4 KB row and is broadcast across partitions on-chip
    (gpsimd partition_broadcast), keeping the 512 KB replica off HBM.
  - On-core: stream the 32 MB shard through SBUF in [128, nb*1024]
    chunks with a CONTIGUOUS per-partition layout (partition p of a
    chunk holds nb consecutive rows -> 16 KB contiguous HBM reads per
    partition, 16 KB DMA descriptors, ~405 GB/s observed).
    All chunks keep the full 128-partition shape: partition-sliced DMAs
    collapse onto 4 of the 16 SDMA rings (~1/4 bandwidth -- measured).
  - Compute: the DVE multiplies each chunk by hid IN PLACE (f32
    tensor_tensor is 1 elem/lane/cycle -> 4.42 us per 4-block chunk vs
    the 5.2-5.9 us DMA pace).  Per-block row-sums are split between the
    Scalar engine (activation Identity + accum_out, 1.22 us/block
    sustained) and the DVE (reduce_sum, 1.22 us/block): scalar takes
    most blocks, the DVE takes a few mid-stream ones plus the tapered
    tail blocks interleaved with its final muls, so neither engine
    holds a backlog when the stream ends.
  - Device returns raw energies; softmax runs on the host in float64
    over all 65536 gathered energies.  No on-device softmax -> no
    ACT_TABLE_LOAD, minimal post-stream tail.
  (tensor_tensor_reduce would fuse mul+reduce in one DVE op but crashes
  the execution unit on this runtime path -- probed 2026-08-08.)
"""

import os
import sys
import time

for _p in ("/opt/trn_rl_repo", "/root/.axon_site/_ro/trn_rl_repo"):
    if os.path.isdir(_p) and _p not in sys.path:
        sys.path.append(_p)

import numpy as np

import concourse.tile as tile
from concourse import bacc, mybir
from concourse.bass_utils import run_bass_kernel_spmd

S = 65536
H = 1024
N_CORES = 8
SC = S // N_CORES          # 8192 rows per core
P = 128                    # partitions
NT = SC // P               # 64 blocks of 128 rows per core
GMAX = 4                   # max blocks per DMA chunk (2 MB; 8-block
                           # chunks hit a DVE efficiency cliff)

# chunk sizes in blocks; tapered at the end (shorter post-DMA tail)
CHUNKS = [4] * 14 + [2, 2, 2, 1, 1]
assert sum(CHUNKS) == NT

INP_BUFS = 7

_DT = mybir.dt.float32


def _vector_blocks(g, nb):
    """Block offsets (within chunk g) computed entirely on the DVE by a
    fused scalar_tensor_tensor (mul+accum in ONE 1.25us pass -- the
    only fused mul+reduce that works on this runtime; the TTR op
    crashes).  All other blocks: DVE multiply + Scalar-engine row-sum.
    STT owns the taper so the endgame has no cross-engine handoff."""
    if nb == 4:
        return [3] if g in (3, 9) else []
    if nb == 2:
        return [1]            # taper 2-block chunks: [scalar, STT]
    return [0]                # final 1-block chunks: STT


def _build_nc():
    nc = bacc.Bacc("TRN2", target_bir_lowering=False, debug=False,
                   enable_asserts=False, num_devices=N_CORES)
    enc = nc.dram_tensor("enc", [SC, H], _DT, kind="ExternalInput")
    hid = nc.dram_tensor("hid", [1, H], _DT, kind="ExternalInput")
    n_v = sum(len(_vector_blocks(g, nb)) for g, nb in enumerate(CHUNKS))
    out_s = nc.dram_tensor("out_s", [P, NT], _DT, kind="ExternalOutput")
    out_v = nc.dram_tensor("out_v", [P, n_v], _DT, kind="ExternalOutput")

    with tile.TileContext(nc) as tc:
        with (
            tc.tile_pool(name="inp", bufs=INP_BUFS) as inp_pool,
            tc.tile_pool(name="small", bufs=1) as small,
        ):
            hidrep = small.tile([P, H], _DT)

            # separate tiles per engine so the Tile tracker never makes
            # one engine's energy writes wait on the other's
            energies_s = small.tile([P, NT], _DT)
            energies_v = small.tile([P, max(n_v, 1)], _DT)
            # the DVE-reduced columns of energies_s are never written on
            # device (host overwrites them from out_v); zero once so the
            # final out_s DMA reads initialized memory
            nc.gpsimd.memset(energies_s[:], 0.0)

            vcol = 0
            blk = 0
            for g, nb in enumerate(CHUNKS):
                r0 = blk * P
                t_in = inp_pool.tile([P, GMAX * H], _DT, tag="t_in")
                # partition p <- rows [r0 + p*nb, r0 + (p+1)*nb):
                # 4*nb KB contiguous per partition.  All chunk DMAs
                # issue from the Sync sequencer: scalar-issued DMAs
                # serialize behind the Scalar engine's ACTIVATE queue
                # and starve the stream (measured +15 us).
                nc.sync.dma_start(
                    t_in[:, :nb * H].rearrange("p (b h) -> p b h", h=H),
                    enc.ap()[r0:r0 + nb * P, :].rearrange(
                        "(p b) h -> p b h", p=P),
                )
                if g == 0:
                    # 4 KB hid row + on-chip broadcast on the idle
                    # GpSimd engine: keeps the 512 KB replica off the
                    # HBM stream (and off the slow SDMA engine 15)
                    nc.scalar.dma_start(hidrep[0:1, :], hid.ap())
                    nc.gpsimd.partition_broadcast(hidrep[:], hidrep[0:1, :])
                vblocks = _vector_blocks(g, nb)
                n_mul = nb - len(vblocks)   # STT blocks are the last ones
                # multiply scalar-owned blocks in <=2-block pieces: the
                # Scalar engine's row-sums for the early blocks start
                # ~2 us sooner than with one whole-chunk mul
                for m0 in range(0, n_mul, 2):
                    mb = min(2, n_mul - m0)
                    hid_bc = hidrep[:].rearrange(
                        "p (o h) -> p o h", o=1).broadcast_to((P, mb, H))
                    nc.vector.tensor_mul(
                        t_in[:, m0 * H:(m0 + mb) * H].rearrange(
                            "p (b h) -> p b h", h=H),
                        t_in[:, m0 * H:(m0 + mb) * H].rearrange(
                            "p (b h) -> p b h", h=H),
                        hid_bc,
                    )
                for j in range(nb):
                    seg = t_in[:, j * H:(j + 1) * H]
                    if j in vblocks:
                        # fused mul+accum on the DVE, one pass
                        nc.vector.scalar_tensor_tensor(
                            seg, seg, 1.0, hidrep[:],
                            op0=mybir.AluOpType.mult,
                            op1=mybir.AluOpType.mult,
                            accum_out=energies_v[:, vcol:vcol + 1],
                        )
                        vcol += 1
                    else:
                        nc.scalar.activation(
                            seg, seg,
                            mybir.ActivationFunctionType.Identity,
                            accum_out=energies_s[:, blk + j:blk + j + 1],
                        )
                blk += nb

            nc.sync.dma_start(out_s.ap(), energies_s[:])
            nc.sync.dma_start(out_v.ap(), energies_v[:, :n_v])
    nc.compile()
    return nc


_NC_CACHE = None


def _get_nc():
    global _NC_CACHE
    if _NC_CACHE is None:
        _NC_CACHE = _build_nc()
    return _NC_CACHE


def run_device(hidden, encoder_outputs, **spmd_kwargs):
    """Run the per-core kernels; returns (list of per-core result dicts,
    BassKernelResults)."""
    hidden = np.asarray(hidden, dtype=np.float32)
    encoder_outputs = np.asarray(encoder_outputs, dtype=np.float32)
    hid_row = np.ascontiguousarray(hidden.reshape(1, H))
    in_maps = [
        {
            "enc": np.ascontiguousarray(encoder_outputs[c * SC:(c + 1) * SC]),
            "hid": hid_row,
        }
        for c in range(N_CORES)
    ]
    # The axon-proxied runtime occasionally reports the accelerator as
    # unrecoverable and then recovers on the next attempt; retry.
    last_err = None
    for attempt in range(3):
        try:
            res = run_bass_kernel_spmd(
                _get_nc(), in_maps, list(range(N_CORES)), **spmd_kwargs
            )
            return res.results, res
        except Exception as e:  # noqa: BLE001
            last_err = e
            time.sleep(2.0)
    raise last_err


def _maps():
    """(vcols, perm): vcols[i] = global block column of the i-th DVE
    reduce; perm[s_local] = flat index into the merged [P, NT] energies."""
    vcols = []
    blk = 0
    for g, nb in enumerate(CHUNKS):
        for j in _vector_blocks(g, nb):
            vcols.append(blk + j)
        blk += nb
    perm = np.empty(SC, dtype=np.int64)
    blk = 0
    for nb in CHUNKS:
        r0 = blk * P
        for p in range(P):
            base = r0 + p * nb
            for j in range(nb):
                perm[base + j] = p * NT + (blk + j)
        blk += nb
    return np.array(vcols), perm


_VCOLS, _PERM = _maps()


def combine(results):
    """Host-side softmax over the gathered energies -> [1, 1, S] f32."""
    es = []
    for r in results:
        E = r["out_s"].copy()
        E[:, _VCOLS] = r["out_v"]
        es.append(E.reshape(P * NT)[_PERM])
    e = np.concatenate(es).astype(np.float64)
    e -= e.max()
    x = np.exp(e)
    attn = x / x.sum()
    return attn.astype(np.float32)[None, None, :]


def kernel(hidden, encoder_outputs):
    results, _ = run_device(hidden, encoder_outputs)
    return combine(results)
